# revision 18
# baseline (speedup 1.0000x reference)
"""Multi-head attention (B=2, S=2048, D=1024, H=16) on 8 Trainium2 NeuronCores.

Sharding: data-parallel over batch (cores 0-3 -> b=0, cores 4-7 -> b=1) and
tensor-parallel over heads (4 heads per core, with the matching 256-row slices
of Wq/Wk/Wv and 256-column slice of Wo). Each core computes its
[4, S, S] attention-probability block (stored transposed, [sk, sq]) and a
[1024, S] partial of the output projection; the host transposes/concats the
probability blocks and sums the output partials (+bo).

Everything inside one core runs in the "transposed" orientation so the
contraction dim always lands on SBUF partitions:
  xT (via PE transpose) -> QT/KT = WsT.T @ xT, V natural = xT.T @ WvT
  E.T[sk,sq] = exp((KT_h)^T.T @ QT_h / 8 + maskbias)    (ACT, PSUM->SBUF)
  [ctxT | sumE] = [V_h | 1].T @ E.T                     (ones-column trick)
  P.T = E.T * (1/sumE  broadcast via K=1 matmul)        (DVE)
  outT_partial = WoT_local.T @ (ctxT * 1/sumE)
"""

import os
import re
import sys
from contextlib import ExitStack

import numpy as np

for _p in ("/opt/trn_rl_repo", "/root/.axon_site/_ro/trn_rl_repo"):
    if os.path.isdir(_p) and _p not in sys.path:
        sys.path.append(_p)

import bass_rust
import concourse.bass as bass
import concourse.tile as _tile
from concourse import mybir
from concourse.bass_utils import run_bass_kernel_spmd
from concourse.masks import make_identity
from concourse.vector_clock import ScopedClock

# ---------------------------------------------------------------------------
# Workaround: this walrus build rejects >1 sync-wait on a CTRL instruction,
# but TileContext's tail drain waits on the full global clock. Split those
# waits across preceding NOPs (one proc each) via the rust add_sem_waits path.
# ---------------------------------------------------------------------------


def _clock_vals(vc):
    m = re.match(r"VectorClock\(\[(.*)\]\)", repr(vc))
    return [int(s) for s in m.group(1).split(",")] if m.group(1).strip() else []


def _patched_drain_and_barrier(self, tick_clock, wait_clock):
    nc = self.nc
    g = tick_clock.global_clock
    for i, v in enumerate(_clock_vals(g)):
        if v > 0:
            nop = nc.sync.nop()
            pc = bass_rust.VectorClock()
            pc.require_at_least(i, v)
            wait_clock.add_sem_waits(nop.ins, ScopedClock({None: pc}))
    nc.sync.drain()
    nc.all_engine_barrier()
    assert self.sems is not None
    popped = nc._tile_sem_poison_stack.pop()
    assert popped is self._sem_poison
    nc.clear_and_free_semaphores(list(self.sems.allocated().values()))
    nc.all_engine_barrier()


_tile.TileContext._drain_and_barrier = _patched_drain_and_barrier

# Capture the Tile scheduler's cost-model makespan (ns) for reporting: the
# axon client cannot host NTFF profiling, so this model estimate is the best
# available per-run hardware execution time figure.
predicted_exec_ns = None
_orig_schedule_block = _tile.TileContext.schedule_block


def _capture_schedule_block(self, *a, **k):
    global predicted_exec_ns
    out = _orig_schedule_block(self, *a, **k)
    try:
        predicted_exec_ns = int(out[1].time)
    except Exception:
        pass
    return out


_tile.TileContext.schedule_block = _capture_schedule_block


def _split_multi_waits(nc):
    """Rebuild the module so no instruction carries more than one sync-wait
    (or more than one sync-update): excess waits move to NoOps inserted just
    before the instruction on the same engine; excess updates move to NoOps
    just after. bb.instructions handles are live, so mutating them and
    reconstructing the containers persists the changes."""
    m = nc.m
    f = m.functions[0]
    new_blocks = []
    k = 0
    for bb in f.blocks:
        out = []
        for inst in bb.instructions:
            si = inst.sync_info
            pre, post = [], []
            if si is not None and (
                (si.on_wait and len(si.on_wait) > 1)
                or (si.on_update and len(si.on_update) > 1)
            ):
                waits = list(si.on_wait or [])
                updates = list(si.on_update or [])
                for w in waits[:-1]:
                    k += 1
                    nop = bass_rust.InstNoOp()
                    nop.name = f"SWX-{k}"
                    nop.engine = inst.engine
                    nop.sync_info = bass_rust.SyncInfo(on_wait=[w], on_update=[])
                    pre.append(nop)
                for u in updates[1:]:
                    k += 1
                    nop = bass_rust.InstNoOp()
                    nop.name = f"SWX-{k}"
                    nop.engine = inst.engine
                    nop.sync_info = bass_rust.SyncInfo(on_wait=[], on_update=[u])
                    post.append(nop)
                inst.sync_info = bass_rust.SyncInfo(
                    on_wait=waits[-1:], on_update=updates[:1]
                )
            out.extend(pre)
            out.append(inst)
            out.extend(post)
        nb = bass_rust.BasicBlock(name=bb.name, instructions=out)
        for flag in ("IsExit", "IsLoopEntry", "IsPredicated"):
            try:
                setattr(nb, flag, getattr(bb, flag))
            except Exception:
                pass
        new_blocks.append(nb)
    nf = bass_rust.Function(name=f.name, attributes=f.attributes, blocks=new_blocks)
    nf.set_allocations_from_list(f.allocations)
    nm = bass_rust.Module(version=m.version, arch=m.arch, functions=[nf])
    for attr in (
        "attributes",
        "queues",
        "ant_sem_names",
        "ant_custom_dve_ops",
        "ant_interned_notif",
        "call_to_physical_memlocs",
    ):
        try:
            setattr(nm, attr, getattr(m, attr))
        except Exception:
            pass
    nc.m = nm
    return k

F32 = mybir.dt.float32
I32 = mybir.dt.int32
EXP = mybir.ActivationFunctionType.Exp

S = 2048  # sequence length
D = 1024  # model dim
HL = 4  # heads per core
DH = 64  # head dim
ML = HL * DH  # local model dim (256)
NCORES = 8
INV_SCALE = 1.0 / 8.0  # 1/sqrt(DH)

# Heavy-matmul operand dtype: fp32 is exact but runs the PE at 1/4 rate
# (two half-speed passes); float32r is single-pass. Small K=1 matmuls that
# directly scale the softmax (bias/mask/reciprocal broadcasts) stay fp32.
MM_DT = os.environ.get("KERNEL_MM_DT", "fp32r")
NORM_MOD = tuple(
    int(x) for x in os.environ.get("KERNEL_NORM_MOD", "3,1").split(",")
)


def _build_program() -> bass.Bass:
    nc = bass.Bass()

    xq = nc.dram_tensor("xq", [S, D], F32, kind="ExternalInput")
    xk = nc.dram_tensor("xk", [S, D], F32, kind="ExternalInput")
    xv = nc.dram_tensor("xv", [S, D], F32, kind="ExternalInput")
    mask = nc.dram_tensor("mask", [1, S], I32, kind="ExternalInput")
    wq = nc.dram_tensor("wq", [ML, D], F32, kind="ExternalInput")
    wk = nc.dram_tensor("wk", [ML, D], F32, kind="ExternalInput")
    wv = nc.dram_tensor("wv", [ML, D], F32, kind="ExternalInput")
    wo = nc.dram_tensor("wo", [D, ML], F32, kind="ExternalInput")
    bq = nc.dram_tensor("bq", [1, ML], F32, kind="ExternalInput")
    bk = nc.dram_tensor("bk", [1, ML], F32, kind="ExternalInput")
    bv = nc.dram_tensor("bv", [1, ML], F32, kind="ExternalInput")
    attn = nc.dram_tensor("attn", [HL, S, S], F32, kind="ExternalOutput")
    outp = nc.dram_tensor("outp", [D, S], F32, kind="ExternalOutput")

    with _tile.TileContext(nc) as tc, ExitStack() as ctx:
        persist = ctx.enter_context(tc.tile_pool(name="persist", bufs=1))

        MMD = mybir.dt.float32r if MM_DT == "fp32r" else F32
        ident = persist.tile([128, 128], F32, tag="ident", name="ident")
        make_identity(nc, ident[:])
        ones1 = persist.tile([1, 128], F32, tag="ones1", name="ones1")
        nc.vector.memset(ones1[:], 1.0)

        # persistent per-core tensors
        qt = [persist.tile([128, S], MMD, tag=f"qt{t}", name=f"qt{t}") for t in range(2)]
        kt = [persist.tile([128, S], MMD, tag=f"kt{t}", name=f"kt{t}") for t in range(2)]
        vsb = [persist.tile([128, HL * 65], MMD, tag=f"vsb{i}", name=f"vsb{i}") for i in range(16)]
        ctxT = [persist.tile([128, S], MMD, tag=f"ctxT{t}", name=f"ctxT{t}") for t in range(2)]
        wosT = [persist.tile([128, D], MMD, tag=f"wosT{t}", name=f"wosT{t}") for t in range(2)]
        bqT = persist.tile([128, 2], F32, tag="bqT", name="bqT")
        bkT = persist.tile([128, 2], F32, tag="bkT", name="bkT")
        bvb = persist.tile([128, ML], F32, tag="bvb", name="bvb")
        mb = persist.tile([128, 16], F32, tag="mb", name="mb")

        # ------------------------------------------------------------------
        # Phase A: weights/biases/mask prep, x transposes, QKV projections
        # ------------------------------------------------------------------
        with tc.tile_pool(name="prep", bufs=1) as prep, tc.tile_pool(
            name="prep_ps", bufs=1, space="PSUM"
        ) as pap:

            def vec_row(dram, ncols):
                t = prep.tile([1, ncols], F32, tag="vrow", bufs=2, name="vrow")
                nc.sync.dma_start(out=t[:], in_=dram[:])
                return t

            # bq/bk -> per-partition [128,1] columns of bqT/bkT
            for bias_dram, bias_t in ((bq, bqT), (bk, bkT)):
                row = vec_row(bias_dram, ML)
                for m in range(2):
                    ps = pap.tile([128, 1], F32, tag="psv", bufs=2, name="psv")
                    nc.tensor.matmul(
                        ps[:],
                        lhsT=row[0:1, m * 128 : (m + 1) * 128],
                        rhs=ones1[0:1, 0:1],
                        start=True,
                        stop=True,
                    )
                    nc.vector.tensor_copy(bias_t[:, m : m + 1], ps[:])

            # bv broadcast to all 128 partitions: ones[1,128].T @ bv[1,256]
            row = vec_row(bv, ML)
            psb = pap.tile([128, ML], F32, tag="psbv", bufs=1, name="psbv")
            nc.tensor.matmul(
                psb[:], lhsT=ones1[0:1, :], rhs=row[0:1, :], start=True, stop=True
            )
            nc.vector.tensor_copy(bvb[:], psb[:])

            # mask -> fp32 additive bias (-1e9 where 0), transposed per sk tile
            mrow_i = prep.tile([1, S], I32, tag="mrowi", name="mrowi")
            nc.sync.dma_start(out=mrow_i[:], in_=mask[:])
            mrow_f = prep.tile([1, S], F32, tag="mrowf", name="mrowf")
            nc.vector.tensor_copy(mrow_f[:], mrow_i[:])
            nc.vector.tensor_scalar(
                out=mrow_f[:],
                in0=mrow_f[:],
                scalar1=-1.0,
                scalar2=1.0e9,
                op0=mybir.AluOpType.add,
                op1=mybir.AluOpType.mult,
            )
            for sk in range(16):
                ps = pap.tile([128, 1], F32, tag="psv", bufs=2, name="psv")
                nc.tensor.matmul(
                    ps[:],
                    lhsT=mrow_f[0:1, sk * 128 : (sk + 1) * 128],
                    rhs=ones1[0:1, 0:1],
                    start=True,
                    stop=True,
                )
                nc.vector.tensor_copy(mb[:, sk : sk + 1], ps[:])

            # weight transposes: wq/wk/wv [256,1024] -> wT lists of [128, 256]
            def load_and_transpose_w(dram):
                wsb = [pa.tile([128, D], F32, tag="wsb", bufs=3, name="wsb") for _ in range(2)]
                for m in range(2):
                    nc.sync.dma_start(
                        out=wsb[m][:], in_=dram[m * 128 : (m + 1) * 128, :]
                    )
                wT = [pa.tile([128, ML], MMD, tag=f"wT{i}", name=f"wT{i}") for i in range(8)]
                for i in range(8):
                    ps = pap.tile([128, 512], F32, tag="pst", bufs=3, name="pst")
                    for m in range(2):
                        nc.tensor.transpose(
                            ps[:, m * 128 : (m + 1) * 128],
                            wsb[m][:, i * 128 : (i + 1) * 128],
                            ident[:],
                        )
                    nc.vector.tensor_copy(wT[i][:], ps[:, 0:ML])
                return wT

            # wo [1024, 256] -> wosT 2x[128, 1024]
            wo_sb = [prep.tile([128, ML], F32, tag="wosb", bufs=8, name="wosb") for _ in range(8)]
            for oc in range(8):
                nc.sync.dma_start(
                    out=wo_sb[oc][:], in_=wo[oc * 128 : (oc + 1) * 128, :]
                )
            for t2 in range(2):
                for oc4 in range(2):
                    ps = pap.tile([128, 512], F32, tag="pst", bufs=3, name="pst")
                    for c in range(4):
                        oc = oc4 * 4 + c
                        nc.tensor.transpose(
                            ps[:, c * 128 : (c + 1) * 128],
                            wo_sb[oc][:, t2 * 128 : (t2 + 1) * 128],
                            ident[:],
                        )
                    nc.scalar.activation(
                        wosT[t2][:, oc4 * 512 : (oc4 + 1) * 512],
                        ps[:],
                        mybir.ActivationFunctionType.Copy,
                    )

        with tc.tile_pool(name="pha", bufs=1) as pa, tc.tile_pool(
            name="pha_ps", bufs=1, space="PSUM"
        ) as pap:
            # x transposes + projections, one input tensor at a time
            xT = [pa.tile([128, S], MMD, tag=f"xT{i}", name=f"xT{i}") for i in range(8)]
            for which, (xdram, wdram) in enumerate(((xq, wq), (xk, wk), (xv, wv))):
                wT = load_and_transpose_w(wdram)
                # transpose x into xT (halves of 8 s-chunks)
                for half in range(2):
                    xs = []
                    for scl in range(8):
                        t = pa.tile([128, D], F32, tag="xs", bufs=10, name="xs")
                        sc = half * 8 + scl
                        ld_eng = nc.gpsimd if scl % 2 == 0 else nc.sync
                        ld_eng.dma_start(
                            out=t[:], in_=xdram[sc * 128 : (sc + 1) * 128, :]
                        )
                        xs.append(t)
                    for i in range(8):
                        for q4 in range(2):
                            ps = pap.tile([128, 512], F32, tag="pst", bufs=3, name="pst")
                            for c in range(4):
                                scl = q4 * 4 + c
                                nc.tensor.transpose(
                                    ps[:, c * 128 : (c + 1) * 128],
                                    xs[scl][:, i * 128 : (i + 1) * 128],
                                    ident[:],
                                )
                            dst = xT[i][
                                :, (half * 8 + q4 * 4) * 128 : (half * 8 + q4 * 4 + 4) * 128
                            ]
                            if (i + q4) % 2 == 0:
                                nc.vector.tensor_copy(dst, ps[:])
                            else:
                                nc.scalar.activation(
                                    dst, ps[:], mybir.ActivationFunctionType.Copy
                                )
                if which < 2:  # Q or K -> QT/KT [256, S] with bias
                    dstT, biasT = (qt, bqT) if which == 0 else (kt, bkT)
                    for m in range(2):
                        for j in range(4):
                            pp = pap.tile([128, 512], F32, tag="psp", bufs=2, name="psp")
                            for i in range(8):
                                nc.tensor.matmul(
                                    pp[:],
                                    lhsT=wT[i][:, m * 128 : (m + 1) * 128],
                                    rhs=xT[i][:, j * 512 : (j + 1) * 512],
                                    start=(i == 0),
                                    stop=(i == 7),
                                )
                            nc.vector.tensor_scalar_add(
                                dstT[m][:, j * 512 : (j + 1) * 512],
                                pp[:],
                                biasT[:, m : m + 1],
                            )
                else:  # V natural [S, 256] + bias, packed as [V_h | 1] groups
                    for i16 in range(16):
                        nc.vector.memset(vsb[i16][:].bitcast(F32), 1.0)
                    for sc in range(16):
                        pp = pap.tile([128, 512], F32, tag="psp", bufs=2, name="psp")
                        for i in range(8):
                            nc.tensor.matmul(
                                pp[:, 0:ML],
                                lhsT=xT[i][:, sc * 128 : (sc + 1) * 128],
                                rhs=wT[i][:],
                                start=(i == 0),
                                stop=(i == 7),
                            )
                        for hh in range(HL):
                            nc.vector.tensor_add(
                                vsb[sc][:, hh * 65 : hh * 65 + 64],
                                pp[:, hh * 64 : (hh + 1) * 64],
                                bvb[:, hh * 64 : (hh + 1) * 64],
                            )

        # ------------------------------------------------------------------
        # Phase B: attention per (head, sq-block)
        # ------------------------------------------------------------------
        with tc.tile_pool(name="phb", bufs=1) as pb, tc.tile_pool(
            name="phb_ps", bufs=1, space="PSUM"
        ) as pbp:
            for h in range(HL):
                t, po = h // 2, (h % 2) * 64
                ets = None
                for j in range(4):
                    if j % 2 == 0:
                        # each et tile spans two sq-blocks so the attn store
                        # is one 1 MB DMA per (sk, block-pair)
                        ets = [
                            pb.tile([128, 1024], MMD, tag="et", bufs=28, name="et")
                            for _ in range(16)
                        ]
                    col = (j % 2) * 512
                    av = pbp.tile([65, 512], F32, tag="av", bufs=2, name="av")
                    for sk in range(16):
                        sc = pbp.tile([128, 512], F32, tag="sc", bufs=4, name="sc")
                        nc.tensor.matmul(
                            sc[:],
                            lhsT=kt[t][po : po + 64, sk * 128 : (sk + 1) * 128],
                            rhs=qt[t][po : po + 64, j * 512 : (j + 1) * 512],
                            start=True,
                            stop=True,
                        )
                        nc.scalar.activation(
                            ets[sk][:, col : col + 512],
                            sc[:],
                            EXP,
                            bias=mb[:, sk : sk + 1],
                            scale=INV_SCALE,
                        )
                    for sk in range(16):
                        nc.tensor.matmul(
                            av[:],
                            lhsT=vsb[sk][:, h * 65 : (h + 1) * 65],
                            rhs=ets[sk][:, col : col + 512],
                            start=(sk == 0),
                            stop=(sk == 15),
                        )
                    r = pb.tile([1, 512], F32, tag="r", bufs=2, name="r")
                    nc.vector.reciprocal(r[:], av[64:65, :])
                    rbp = pbp.tile([128, 512], F32, tag="rb", bufs=2, name="rb")
                    nc.tensor.matmul(
                        rbp[:], lhsT=ones1[0:1, :], rhs=r[:], start=True, stop=True
                    )
                    rbs = pb.tile([128, 512], F32, tag="rbs", bufs=3, name="rbs")
                    nc.vector.tensor_copy(rbs[:], rbp[:])
                    nc.vector.tensor_mul(
                        ctxT[t][po : po + 64, j * 512 : (j + 1) * 512],
                        av[0:64, :],
                        rbs[0:64, :],
                    )
                    for sk in range(16):
                        etb = ets[sk][:, col : col + 512]
                        # normalize split across DVE and GpSimd to balance load
                        if sk % NORM_MOD[0] < NORM_MOD[1]:
                            nc.gpsimd.tensor_mul(etb, etb, rbs[:])
                        else:
                            nc.vector.tensor_mul(etb, etb, rbs[:])
                        if j % 2 == 1:
                            st_eng = nc.gpsimd if sk % 4 == 3 else nc.sync
                            st_eng.dma_start(
                                out=attn[
                                    h,
                                    sk * 128 : (sk + 1) * 128,
                                    (j - 1) * 512 : (j + 1) * 512,
                                ],
                                in_=ets[sk][:].bitcast(F32),
                            )

        # ------------------------------------------------------------------
        # Phase C: output projection partial outT = wosT.T @ ctxT
        # ------------------------------------------------------------------
        with tc.tile_pool(name="phc", bufs=1) as pc, tc.tile_pool(
            name="phc_ps", bufs=1, space="PSUM"
        ) as pcp:
            for m in range(8):
                for j in range(4):
                    op = pcp.tile([128, 512], F32, tag="op", bufs=2, name="op")
                    for t2 in range(2):
                        nc.tensor.matmul(
                            op[:],
                            lhsT=wosT[t2][:, m * 128 : (m + 1) * 128],
                            rhs=ctxT[t2][:, j * 512 : (j + 1) * 512],
                            start=(t2 == 0),
                            stop=(t2 == 1),
                        )
                    ob = pc.tile([128, 512], F32, tag="ob", bufs=3, name="ob")
                    nc.vector.tensor_copy(ob[:], op[:])
                    nc.sync.dma_start(
                        out=outp[m * 128 : (m + 1) * 128, j * 512 : (j + 1) * 512],
                        in_=ob[:],
                    )

    n_split = _split_multi_waits(nc)
    if os.environ.get("KERNEL_DEBUG"):
        print(f"[kernel] split {n_split} excess sync waits/updates onto NoOps")
    return nc


_cached_nc = None
last_results = None


def _get_program():
    global _cached_nc
    if _cached_nc is None:
        _cached_nc = _build_program()
    return _cached_nc


def kernel(query, key, value, key_padding_mask, Wq, bq, Wk, bk, Wv, bv, Wo, bo):
    query = np.asarray(query, np.float32)
    key = np.asarray(key, np.float32)
    value = np.asarray(value, np.float32)
    key_padding_mask = np.asarray(key_padding_mask, np.int32)
    Wq, bq = np.asarray(Wq, np.float32), np.asarray(bq, np.float32)
    Wk, bk = np.asarray(Wk, np.float32), np.asarray(bk, np.float32)
    Wv, bv = np.asarray(Wv, np.float32), np.asarray(bv, np.float32)
    Wo, bo = np.asarray(Wo, np.float32), np.asarray(bo, np.float32)

    nc = _get_program()
    in_maps = []
    for c in range(NCORES):
        b, g = c // 4, c % 4
        r0 = g * ML
        in_maps.append(
            {
                "xq": np.ascontiguousarray(query[b]),
                "xk": np.ascontiguousarray(key[b]),
                "xv": np.ascontiguousarray(value[b]),
                "mask": np.ascontiguousarray(key_padding_mask[b].reshape(1, S)),
                "wq": np.ascontiguousarray(Wq[r0 : r0 + ML, :]),
                "wk": np.ascontiguousarray(Wk[r0 : r0 + ML, :]),
                "wv": np.ascontiguousarray(Wv[r0 : r0 + ML, :]),
                "wo": np.ascontiguousarray(Wo[:, r0 : r0 + ML]),
                "bq": np.ascontiguousarray(bq[r0 : r0 + ML].reshape(1, ML)),
                "bk": np.ascontiguousarray(bk[r0 : r0 + ML].reshape(1, ML)),
                "bv": np.ascontiguousarray(bv[r0 : r0 + ML].reshape(1, ML)),
            }
        )

    trace = bool(os.environ.get("KERNEL_TRACE"))
    res = run_bass_kernel_spmd(
        nc, in_maps, core_ids=list(range(NCORES)), trace=trace
    )
    if trace:
        global last_results
        last_results = res

    attn = np.empty((2, 16, S, S), np.float32)
    out = np.zeros((2, S, D), np.float32)
    for c in range(NCORES):
        b, g = c // 4, c % 4
        attn[b, g * HL : (g + 1) * HL] = res.results[c]["attn"].transpose(0, 2, 1)
        out[b] += res.results[c]["outp"].T
    out += bo.reshape(1, 1, D)
    return out, attn


# revision 23
# speedup vs baseline: 1.0406x; 1.0406x over previous
"""Multi-head attention (B=2, S=2048, D=1024, H=16) on 8 Trainium2 NeuronCores.

Sharding: data-parallel over batch (cores 0-3 -> b=0, cores 4-7 -> b=1) and
tensor-parallel over heads (4 heads per core, with the matching 256-row slices
of Wq/Wk/Wv and 256-column slice of Wo). Each core computes its
[4, S, S] attention-probability block (stored transposed, [sk, sq]) and a
[1024, S] partial of the output projection; the host transposes/concats the
probability blocks and sums the output partials (+bo).

Everything inside one core runs in the "transposed" orientation so the
contraction dim always lands on SBUF partitions:
  xT (via PE transpose) -> QT/KT = WsT.T @ xT, V natural = xT.T @ WvT
  E.T[sk,sq] = exp((KT_h)^T.T @ QT_h / 8 + maskbias)    (ACT, PSUM->SBUF)
  [ctxT | sumE] = [V_h | 1].T @ E.T                     (ones-column trick)
  P.T = E.T * (1/sumE  broadcast via K=1 matmul)        (DVE)
  outT_partial = WoT_local.T @ (ctxT * 1/sumE)
"""

import os
import re
import sys
from contextlib import ExitStack

import numpy as np

for _p in ("/opt/trn_rl_repo", "/root/.axon_site/_ro/trn_rl_repo"):
    if os.path.isdir(_p) and _p not in sys.path:
        sys.path.append(_p)

import bass_rust
import concourse.bass as bass
import concourse.tile as _tile
from concourse import mybir
from concourse.bass_utils import run_bass_kernel_spmd
from concourse.masks import make_identity
from concourse.vector_clock import ScopedClock

# ---------------------------------------------------------------------------
# Workaround: this walrus build rejects >1 sync-wait on a CTRL instruction,
# but TileContext's tail drain waits on the full global clock. Split those
# waits across preceding NOPs (one proc each) via the rust add_sem_waits path.
# ---------------------------------------------------------------------------


def _clock_vals(vc):
    m = re.match(r"VectorClock\(\[(.*)\]\)", repr(vc))
    return [int(s) for s in m.group(1).split(",")] if m.group(1).strip() else []


def _patched_drain_and_barrier(self, tick_clock, wait_clock):
    nc = self.nc
    g = tick_clock.global_clock
    for i, v in enumerate(_clock_vals(g)):
        if v > 0:
            nop = nc.sync.nop()
            pc = bass_rust.VectorClock()
            pc.require_at_least(i, v)
            wait_clock.add_sem_waits(nop.ins, ScopedClock({None: pc}))
    nc.sync.drain()
    nc.all_engine_barrier()
    assert self.sems is not None
    popped = nc._tile_sem_poison_stack.pop()
    assert popped is self._sem_poison
    nc.clear_and_free_semaphores(list(self.sems.allocated().values()))
    nc.all_engine_barrier()


_tile.TileContext._drain_and_barrier = _patched_drain_and_barrier

# Capture the Tile scheduler's cost-model makespan (ns) for reporting: the
# axon client cannot host NTFF profiling, so this model estimate is the best
# available per-run hardware execution time figure.
predicted_exec_ns = None
_orig_schedule_block = _tile.TileContext.schedule_block


def _capture_schedule_block(self, *a, **k):
    global predicted_exec_ns
    out = _orig_schedule_block(self, *a, **k)
    try:
        predicted_exec_ns = int(out[1].time)
    except Exception:
        pass
    return out


_tile.TileContext.schedule_block = _capture_schedule_block


def _split_multi_waits(nc):
    """Rebuild the module so no instruction carries more than one sync-wait
    (or more than one sync-update): excess waits move to NoOps inserted just
    before the instruction on the same engine; excess updates move to NoOps
    just after. bb.instructions handles are live, so mutating them and
    reconstructing the containers persists the changes."""
    m = nc.m
    f = m.functions[0]
    new_blocks = []
    k = 0
    for bb in f.blocks:
        out = []
        for inst in bb.instructions:
            si = inst.sync_info
            pre, post = [], []
            if si is not None and (
                (si.on_wait and len(si.on_wait) > 1)
                or (si.on_update and len(si.on_update) > 1)
            ):
                waits = list(si.on_wait or [])
                updates = list(si.on_update or [])
                for w in waits[:-1]:
                    k += 1
                    nop = bass_rust.InstNoOp()
                    nop.name = f"SWX-{k}"
                    nop.engine = inst.engine
                    nop.sync_info = bass_rust.SyncInfo(on_wait=[w], on_update=[])
                    pre.append(nop)
                for u in updates[1:]:
                    k += 1
                    nop = bass_rust.InstNoOp()
                    nop.name = f"SWX-{k}"
                    nop.engine = inst.engine
                    nop.sync_info = bass_rust.SyncInfo(on_wait=[], on_update=[u])
                    post.append(nop)
                inst.sync_info = bass_rust.SyncInfo(
                    on_wait=waits[-1:], on_update=updates[:1]
                )
            out.extend(pre)
            out.append(inst)
            out.extend(post)
        nb = bass_rust.BasicBlock(name=bb.name, instructions=out)
        for flag in ("IsExit", "IsLoopEntry", "IsPredicated"):
            try:
                setattr(nb, flag, getattr(bb, flag))
            except Exception:
                pass
        new_blocks.append(nb)
    nf = bass_rust.Function(name=f.name, attributes=f.attributes, blocks=new_blocks)
    nf.set_allocations_from_list(f.allocations)
    nm = bass_rust.Module(version=m.version, arch=m.arch, functions=[nf])
    for attr in (
        "attributes",
        "queues",
        "ant_sem_names",
        "ant_custom_dve_ops",
        "ant_interned_notif",
        "call_to_physical_memlocs",
    ):
        try:
            setattr(nm, attr, getattr(m, attr))
        except Exception:
            pass
    nc.m = nm
    return k

F32 = mybir.dt.float32
I32 = mybir.dt.int32
EXP = mybir.ActivationFunctionType.Exp

S = 2048  # sequence length
D = 1024  # model dim
HL = 4  # heads per core
DH = 64  # head dim
ML = HL * DH  # local model dim (256)
NCORES = 8
INV_SCALE = 1.0 / 8.0  # 1/sqrt(DH)

# Heavy-matmul operand dtype: fp32 is exact but runs the PE at 1/4 rate
# (two half-speed passes); float32r is single-pass. Small K=1 matmuls that
# directly scale the softmax (bias/mask/reciprocal broadcasts) stay fp32.
MM_DT = os.environ.get("KERNEL_MM_DT", "fp32r")
SC_BUFS = int(os.environ.get("KERNEL_SC_BUFS", "4"))
ST_MOD = tuple(
    int(x) for x in os.environ.get("KERNEL_ST_MOD", "2,1").split(",")
)
NORM_MOD = tuple(
    int(x) for x in os.environ.get("KERNEL_NORM_MOD", "3,1").split(",")
)


def _build_program() -> bass.Bass:
    nc = bass.Bass()

    xq = nc.dram_tensor("xq", [S, D], F32, kind="ExternalInput")
    xk = nc.dram_tensor("xk", [S, D], F32, kind="ExternalInput")
    xv = nc.dram_tensor("xv", [S, D], F32, kind="ExternalInput")
    mask = nc.dram_tensor("mask", [1, S], I32, kind="ExternalInput")
    wq = nc.dram_tensor("wq", [ML, D], F32, kind="ExternalInput")
    wk = nc.dram_tensor("wk", [ML, D], F32, kind="ExternalInput")
    wv = nc.dram_tensor("wv", [ML, D], F32, kind="ExternalInput")
    wo = nc.dram_tensor("wo", [D, ML], F32, kind="ExternalInput")
    bq = nc.dram_tensor("bq", [1, ML], F32, kind="ExternalInput")
    bk = nc.dram_tensor("bk", [1, ML], F32, kind="ExternalInput")
    bv = nc.dram_tensor("bv", [1, ML], F32, kind="ExternalInput")
    attn = nc.dram_tensor("attn", [HL, S, S], F32, kind="ExternalOutput")
    outp = nc.dram_tensor("outp", [D, S], F32, kind="ExternalOutput")

    with _tile.TileContext(nc) as tc, ExitStack() as ctx:
        persist = ctx.enter_context(tc.tile_pool(name="persist", bufs=1))

        MMD = mybir.dt.float32r if MM_DT == "fp32r" else F32
        ident = persist.tile([128, 128], F32, tag="ident", name="ident")
        make_identity(nc, ident[:])
        ones1 = persist.tile([1, 128], F32, tag="ones1", name="ones1")
        nc.vector.memset(ones1[:], 1.0)

        # persistent per-core tensors
        qt = [persist.tile([128, S], MMD, tag=f"qt{t}", name=f"qt{t}") for t in range(2)]
        kt = [persist.tile([128, S], MMD, tag=f"kt{t}", name=f"kt{t}") for t in range(2)]
        vsb = [persist.tile([128, HL * 65], MMD, tag=f"vsb{i}", name=f"vsb{i}") for i in range(16)]
        ctxT = [persist.tile([128, S], MMD, tag=f"ctxT{t}", name=f"ctxT{t}") for t in range(2)]
        wosT = [persist.tile([128, D], MMD, tag=f"wosT{t}", name=f"wosT{t}") for t in range(2)]
        bqT = persist.tile([128, 2], F32, tag="bqT", name="bqT")
        bkT = persist.tile([128, 2], F32, tag="bkT", name="bkT")
        bvb = persist.tile([128, ML], F32, tag="bvb", name="bvb")
        mb = persist.tile([128, 16], F32, tag="mb", name="mb")

        # ------------------------------------------------------------------
        # Phase A: weights/biases/mask prep, x transposes, QKV projections
        # ------------------------------------------------------------------
        with tc.tile_pool(name="prep", bufs=1) as prep, tc.tile_pool(
            name="prep_ps", bufs=1, space="PSUM"
        ) as pap:

            def vec_row(dram, ncols):
                t = prep.tile([1, ncols], F32, tag="vrow", bufs=2, name="vrow")
                nc.sync.dma_start(out=t[:], in_=dram[:])
                return t

            # bq/bk -> per-partition [128,1] columns of bqT/bkT
            for bias_dram, bias_t in ((bq, bqT), (bk, bkT)):
                row = vec_row(bias_dram, ML)
                for m in range(2):
                    ps = pap.tile([128, 1], F32, tag="psv", bufs=2, name="psv")
                    nc.tensor.matmul(
                        ps[:],
                        lhsT=row[0:1, m * 128 : (m + 1) * 128],
                        rhs=ones1[0:1, 0:1],
                        start=True,
                        stop=True,
                    )
                    nc.vector.tensor_copy(bias_t[:, m : m + 1], ps[:])

            # bv broadcast to all 128 partitions: ones[1,128].T @ bv[1,256]
            row = vec_row(bv, ML)
            psb = pap.tile([128, ML], F32, tag="psbv", bufs=1, name="psbv")
            nc.tensor.matmul(
                psb[:], lhsT=ones1[0:1, :], rhs=row[0:1, :], start=True, stop=True
            )
            nc.vector.tensor_copy(bvb[:], psb[:])

            # mask -> fp32 additive bias (-1e9 where 0), transposed per sk tile
            mrow_i = prep.tile([1, S], I32, tag="mrowi", name="mrowi")
            nc.sync.dma_start(out=mrow_i[:], in_=mask[:])
            mrow_f = prep.tile([1, S], F32, tag="mrowf", name="mrowf")
            nc.vector.tensor_copy(mrow_f[:], mrow_i[:])
            nc.vector.tensor_scalar(
                out=mrow_f[:],
                in0=mrow_f[:],
                scalar1=-1.0,
                scalar2=1.0e9,
                op0=mybir.AluOpType.add,
                op1=mybir.AluOpType.mult,
            )
            for sk in range(16):
                ps = pap.tile([128, 1], F32, tag="psv", bufs=2, name="psv")
                nc.tensor.matmul(
                    ps[:],
                    lhsT=mrow_f[0:1, sk * 128 : (sk + 1) * 128],
                    rhs=ones1[0:1, 0:1],
                    start=True,
                    stop=True,
                )
                nc.vector.tensor_copy(mb[:, sk : sk + 1], ps[:])

            # weight transposes: wq/wk/wv [256,1024] -> wT lists of [128, 256]
            def load_and_transpose_w(dram):
                wsb = [pa.tile([128, D], F32, tag="wsb", bufs=3, name="wsb") for _ in range(2)]
                for m in range(2):
                    nc.sync.dma_start(
                        out=wsb[m][:], in_=dram[m * 128 : (m + 1) * 128, :]
                    )
                wT = [pa.tile([128, ML], MMD, tag=f"wT{i}", name=f"wT{i}") for i in range(8)]
                for i in range(8):
                    ps = pap.tile([128, 512], F32, tag="pst", bufs=3, name="pst")
                    for m in range(2):
                        nc.tensor.transpose(
                            ps[:, m * 128 : (m + 1) * 128],
                            wsb[m][:, i * 128 : (i + 1) * 128],
                            ident[:],
                        )
                    nc.vector.tensor_copy(wT[i][:], ps[:, 0:ML])
                return wT

            # wo [1024, 256] -> wosT 2x[128, 1024]
            wo_sb = [prep.tile([128, ML], F32, tag="wosb", bufs=8, name="wosb") for _ in range(8)]
            for oc in range(8):
                nc.sync.dma_start(
                    out=wo_sb[oc][:], in_=wo[oc * 128 : (oc + 1) * 128, :]
                )
            for t2 in range(2):
                for oc4 in range(2):
                    ps = pap.tile([128, 512], F32, tag="pst", bufs=3, name="pst")
                    for c in range(4):
                        oc = oc4 * 4 + c
                        nc.tensor.transpose(
                            ps[:, c * 128 : (c + 1) * 128],
                            wo_sb[oc][:, t2 * 128 : (t2 + 1) * 128],
                            ident[:],
                        )
                    nc.scalar.activation(
                        wosT[t2][:, oc4 * 512 : (oc4 + 1) * 512],
                        ps[:],
                        mybir.ActivationFunctionType.Copy,
                    )

        with tc.tile_pool(name="pha", bufs=1) as pa, tc.tile_pool(
            name="pha_ps", bufs=1, space="PSUM"
        ) as pap:
            # x transposes + projections, one input tensor at a time
            xT = [pa.tile([128, S], MMD, tag=f"xT{i}", name=f"xT{i}") for i in range(8)]
            for which, (xdram, wdram) in enumerate(((xq, wq), (xk, wk), (xv, wv))):
                wT = load_and_transpose_w(wdram)
                # transpose x into xT (halves of 8 s-chunks)
                for half in range(2):
                    xs = []
                    for scl in range(8):
                        t = pa.tile([128, D], F32, tag="xs", bufs=10, name="xs")
                        sc = half * 8 + scl
                        ld_eng = nc.gpsimd if scl % 2 == 0 else nc.sync
                        ld_eng.dma_start(
                            out=t[:], in_=xdram[sc * 128 : (sc + 1) * 128, :]
                        )
                        xs.append(t)
                    for i in range(8):
                        for q4 in range(2):
                            ps = pap.tile([128, 512], F32, tag="pst", bufs=3, name="pst")
                            for c in range(4):
                                scl = q4 * 4 + c
                                nc.tensor.transpose(
                                    ps[:, c * 128 : (c + 1) * 128],
                                    xs[scl][:, i * 128 : (i + 1) * 128],
                                    ident[:],
                                )
                            dst = xT[i][
                                :, (half * 8 + q4 * 4) * 128 : (half * 8 + q4 * 4 + 4) * 128
                            ]
                            if (i + q4) % 2 == 0:
                                nc.vector.tensor_copy(dst, ps[:])
                            else:
                                nc.scalar.activation(
                                    dst, ps[:], mybir.ActivationFunctionType.Copy
                                )
                if which < 2:  # Q or K -> QT/KT [256, S] with bias
                    dstT, biasT = (qt, bqT) if which == 0 else (kt, bkT)
                    for m in range(2):
                        for j in range(4):
                            pp = pap.tile([128, 512], F32, tag="psp", bufs=2, name="psp")
                            for i in range(8):
                                nc.tensor.matmul(
                                    pp[:],
                                    lhsT=wT[i][:, m * 128 : (m + 1) * 128],
                                    rhs=xT[i][:, j * 512 : (j + 1) * 512],
                                    start=(i == 0),
                                    stop=(i == 7),
                                )
                            nc.vector.tensor_scalar_add(
                                dstT[m][:, j * 512 : (j + 1) * 512],
                                pp[:],
                                biasT[:, m : m + 1],
                            )
                else:  # V natural [S, 256] + bias, packed as [V_h | 1] groups
                    for i16 in range(16):
                        nc.vector.memset(vsb[i16][:].bitcast(F32), 1.0)
                    for sc in range(16):
                        pp = pap.tile([128, 512], F32, tag="psp", bufs=2, name="psp")
                        for i in range(8):
                            nc.tensor.matmul(
                                pp[:, 0:ML],
                                lhsT=xT[i][:, sc * 128 : (sc + 1) * 128],
                                rhs=wT[i][:],
                                start=(i == 0),
                                stop=(i == 7),
                            )
                        for hh in range(HL):
                            nc.vector.tensor_add(
                                vsb[sc][:, hh * 65 : hh * 65 + 64],
                                pp[:, hh * 64 : (hh + 1) * 64],
                                bvb[:, hh * 64 : (hh + 1) * 64],
                            )

        # ------------------------------------------------------------------
        # Phase B: attention per (head, sq-block)
        # ------------------------------------------------------------------
        with tc.tile_pool(name="phb", bufs=1) as pb, tc.tile_pool(
            name="phb_ps", bufs=1, space="PSUM"
        ) as pbp:
            for h in range(HL):
                t, po = h // 2, (h % 2) * 64
                ets = None
                for j in range(4):
                    if j % 2 == 0:
                        # each et tile spans two sq-blocks so the attn store
                        # is one 1 MB DMA per (sk, block-pair)
                        ets = [
                            pb.tile([128, 1024], MMD, tag="et", bufs=28, name="et")
                            for _ in range(16)
                        ]
                    col = (j % 2) * 512
                    av = pbp.tile([65, 512], F32, tag="av", bufs=2, name="av")
                    for sk in range(16):
                        sc = pbp.tile([128, 512], F32, tag="sc", bufs=SC_BUFS, name="sc")
                        nc.tensor.matmul(
                            sc[:],
                            lhsT=kt[t][po : po + 64, sk * 128 : (sk + 1) * 128],
                            rhs=qt[t][po : po + 64, j * 512 : (j + 1) * 512],
                            start=True,
                            stop=True,
                        )
                        nc.scalar.activation(
                            ets[sk][:, col : col + 512],
                            sc[:],
                            EXP,
                            bias=mb[:, sk : sk + 1],
                            scale=INV_SCALE,
                        )
                    for sk in range(16):
                        nc.tensor.matmul(
                            av[:],
                            lhsT=vsb[sk][:, h * 65 : (h + 1) * 65],
                            rhs=ets[sk][:, col : col + 512],
                            start=(sk == 0),
                            stop=(sk == 15),
                        )
                    r = pb.tile([1, 512], F32, tag="r", bufs=2, name="r")
                    nc.vector.reciprocal(r[:], av[64:65, :])
                    rbp = pbp.tile([128, 512], F32, tag="rb", bufs=2, name="rb")
                    nc.tensor.matmul(
                        rbp[:], lhsT=ones1[0:1, :], rhs=r[:], start=True, stop=True
                    )
                    rbs = pb.tile([128, 512], F32, tag="rbs", bufs=3, name="rbs")
                    nc.vector.tensor_copy(rbs[:], rbp[:])
                    nc.vector.tensor_mul(
                        ctxT[t][po : po + 64, j * 512 : (j + 1) * 512],
                        av[0:64, :],
                        rbs[0:64, :],
                    )
                    for sk in range(16):
                        etb = ets[sk][:, col : col + 512]
                        # normalize split across DVE and GpSimd to balance load
                        if sk % NORM_MOD[0] < NORM_MOD[1]:
                            nc.gpsimd.tensor_mul(etb, etb, rbs[:])
                        else:
                            nc.vector.tensor_mul(etb, etb, rbs[:])
                        if j % 2 == 1:
                            st_eng = nc.gpsimd if sk % ST_MOD[0] == ST_MOD[1] else nc.sync
                            st_eng.dma_start(
                                out=attn[
                                    h,
                                    sk * 128 : (sk + 1) * 128,
                                    (j - 1) * 512 : (j + 1) * 512,
                                ],
                                in_=ets[sk][:].bitcast(F32),
                            )

        # ------------------------------------------------------------------
        # Phase C: output projection partial outT = wosT.T @ ctxT
        # ------------------------------------------------------------------
        with tc.tile_pool(name="phc", bufs=1) as pc, tc.tile_pool(
            name="phc_ps", bufs=1, space="PSUM"
        ) as pcp:
            for m in range(8):
                for j in range(4):
                    op = pcp.tile([128, 512], F32, tag="op", bufs=2, name="op")
                    for t2 in range(2):
                        nc.tensor.matmul(
                            op[:],
                            lhsT=wosT[t2][:, m * 128 : (m + 1) * 128],
                            rhs=ctxT[t2][:, j * 512 : (j + 1) * 512],
                            start=(t2 == 0),
                            stop=(t2 == 1),
                        )
                    ob = pc.tile([128, 512], F32, tag="ob", bufs=4, name="ob")
                    if (m + j) % 2 == 0:
                        nc.vector.tensor_copy(ob[:], op[:])
                    else:
                        nc.scalar.activation(
                            ob[:], op[:], mybir.ActivationFunctionType.Copy
                        )
                    st_eng = nc.sync if (m + j) % 2 == 0 else nc.gpsimd
                    st_eng.dma_start(
                        out=outp[m * 128 : (m + 1) * 128, j * 512 : (j + 1) * 512],
                        in_=ob[:],
                    )

    n_split = _split_multi_waits(nc)
    if os.environ.get("KERNEL_DEBUG"):
        print(f"[kernel] split {n_split} excess sync waits/updates onto NoOps")
    return nc


_cached_nc = None
last_results = None


def _get_program():
    global _cached_nc
    if _cached_nc is None:
        _cached_nc = _build_program()
    return _cached_nc


def kernel(query, key, value, key_padding_mask, Wq, bq, Wk, bk, Wv, bv, Wo, bo):
    query = np.asarray(query, np.float32)
    key = np.asarray(key, np.float32)
    value = np.asarray(value, np.float32)
    key_padding_mask = np.asarray(key_padding_mask, np.int32)
    Wq, bq = np.asarray(Wq, np.float32), np.asarray(bq, np.float32)
    Wk, bk = np.asarray(Wk, np.float32), np.asarray(bk, np.float32)
    Wv, bv = np.asarray(Wv, np.float32), np.asarray(bv, np.float32)
    Wo, bo = np.asarray(Wo, np.float32), np.asarray(bo, np.float32)

    nc = _get_program()
    in_maps = []
    for c in range(NCORES):
        b, g = c // 4, c % 4
        r0 = g * ML
        in_maps.append(
            {
                "xq": np.ascontiguousarray(query[b]),
                "xk": np.ascontiguousarray(key[b]),
                "xv": np.ascontiguousarray(value[b]),
                "mask": np.ascontiguousarray(key_padding_mask[b].reshape(1, S)),
                "wq": np.ascontiguousarray(Wq[r0 : r0 + ML, :]),
                "wk": np.ascontiguousarray(Wk[r0 : r0 + ML, :]),
                "wv": np.ascontiguousarray(Wv[r0 : r0 + ML, :]),
                "wo": np.ascontiguousarray(Wo[:, r0 : r0 + ML]),
                "bq": np.ascontiguousarray(bq[r0 : r0 + ML].reshape(1, ML)),
                "bk": np.ascontiguousarray(bk[r0 : r0 + ML].reshape(1, ML)),
                "bv": np.ascontiguousarray(bv[r0 : r0 + ML].reshape(1, ML)),
            }
        )

    trace = bool(os.environ.get("KERNEL_TRACE"))
    res = run_bass_kernel_spmd(
        nc, in_maps, core_ids=list(range(NCORES)), trace=trace
    )
    if trace:
        global last_results
        last_results = res

    attn = np.empty((2, 16, S, S), np.float32)
    out = np.zeros((2, S, D), np.float32)
    for c in range(NCORES):
        b, g = c // 4, c % 4
        attn[b, g * HL : (g + 1) * HL] = res.results[c]["attn"].transpose(0, 2, 1)
        out[b] += res.results[c]["outp"].T
    out += bo.reshape(1, 1, D)
    return out, attn
